# revision 7
# baseline (speedup 1.0000x reference)
"""KNN classification kernel for Trainium2 (8 NeuronCores).

Problem: B=1024 queries x N=200000 gallery, D=256, top-10 neighbors,
softmax-weighted one-hot class scores over 50 classes.

Math fold: reference computes gallery = l2norm(train.T, axis=1) -- i.e. each
feature dim d is normalized by ||train[:, d]|| over the FULL gallery. That
scale folds into the query side:
    sim[b, n] = sum_d (q[b,d]/||q[b]||) * train[n,d] / ||train[:,d]||
              = q_scaled[b] . train[n]
so the device kernel is a pure matmul + top-k screen.

Device (per core, gallery sharded along N into 8 x 25000, zero-padded to
25088 = 49 x 512):
  PE: sim tile [128q, 512n] = q_scaled_bf16.T @ gallery_bf16 (2 K=128 steps),
      two tiles packed into one 2-bank PSUM slot [128, 1024]
  DVE: top-8 values per 1024-col region (InstMax) -> cand [1024, 25*8]
Host: screen top-J candidate values -> identify regions -> recompute those
  regions' sims exactly in f64 -> exact top-10 -> softmax -> class scores.
Safety: a true top-10 item's region has region-max >= item value, so the
region ranks <=10 among all regions by top value -- top-J>=16 region
screening provably covers the true top-10 (modulo bf16 noise, which is
~40 sigma below the rank-10/16 value gaps; verified empirically).
"""

import os
import numpy as np

NB_KNN = 10
T = 0.07
NUM_CLASSES = 50
EPS = 1e-12

B, N, D = 1024, 200000, 256
NCORES = 8
NPC = N // NCORES          # 25000 real cols per core
TILE = 512
NPC_PAD = 25088            # 49 * 512
NT = NPC_PAD // TILE       # 49 tiles per core
BLOCKS = [8, 8, 8, 8, 8, 8, 1]   # tiles per DMA block
NREG = 13                  # 12 grouped regions (2048 cols) + 1 single (512)
TOPJ = 16                  # regions screened per query
GROUP = 4                  # psum tiles per DVE max8 region

_CACHE = {}


def _build_bass():
    import concourse.bacc as bacc
    import concourse.tile as tile
    from concourse import mybir

    nc = bacc.Bacc("TRN2")
    bf16 = mybir.dt.bfloat16
    f32 = mybir.dt.float32

    g_d = nc.dram_tensor("g", [2, 128, NPC_PAD], bf16, kind="ExternalInput")
    q_d = nc.dram_tensor("q", [2, 128, B], bf16, kind="ExternalInput")
    cand_d = nc.dram_tensor("cand", [B, NREG * 8], f32, kind="ExternalOutput")

    with tile.TileContext(nc) as tc:
        with tc.tile_pool(name="qp", bufs=1) as qp, \
             tc.tile_pool(name="gp", bufs=2) as gp, \
             tc.tile_pool(name="cp", bufs=8) as cp, \
             tc.tile_pool(name="pp", bufs=1, space="PSUM") as pp:
            q0 = qp.tile([128, B], bf16, tag="q0")
            q1 = qp.tile([128, B], bf16, tag="q1")
            nc.sync.dma_start(out=q0[:], in_=q_d[0])
            nc.sync.dma_start(out=q1[:], in_=q_d[1])

            cands = [cp.tile([128, NREG * 8], f32, tag="cand",
                             name=f"cand{i}") for i in range(8)]

            tbase = 0
            for blk, ntile in enumerate(BLOCKS):
                cw = ntile * TILE
                c0 = tbase * TILE
                g0 = gp.tile([128, cw], bf16, tag=f"g0_{ntile}")
                g1 = gp.tile([128, cw], bf16, tag=f"g1_{ntile}")
                nc.sync.dma_start(out=g0[:], in_=g_d[0][:, c0:c0 + cw])
                nc.sync.dma_start(out=g1[:], in_=g_d[1][:, c0:c0 + cw])
                for bc in range(8):
                    lhs0 = q0[:, bc * 128:(bc + 1) * 128]
                    lhs1 = q1[:, bc * 128:(bc + 1) * 128]
                    for p in range(0, ntile, GROUP):
                        grp = min(GROUP, ntile - p)
                        ps = pp.tile([128, TILE * GROUP], f32,
                                     tag="ps4", bufs=2)
                        for s in range(grp):
                            rsl = slice((p + s) * TILE, (p + s + 1) * TILE)
                            osl = slice(s * TILE, (s + 1) * TILE)
                            nc.tensor.matmul(ps[:, osl], lhs0, g0[:, rsl],
                                             start=True, stop=False)
                            nc.tensor.matmul(ps[:, osl], lhs1, g1[:, rsl],
                                             start=False, stop=True)
                        reg = (tbase + p) // GROUP
                        nc.vector.max(cands[bc][:, reg * 8:(reg + 1) * 8],
                                      ps[:, :TILE * grp])
                tbase += ntile

            for bc in range(8):
                nc.sync.dma_start(
                    out=cand_d[bc * 128:(bc + 1) * 128, :], in_=cands[bc][:])
    if not nc.is_finalized():
        nc.finalize()
    return nc


def _run_device(g_shards, q_packed):
    from concourse.bass_utils import run_bass_kernel_spmd
    if "nc" not in _CACHE:
        _CACHE["nc"] = _build_bass()
    nc = _CACHE["nc"]
    in_maps = [{"g": g_shards[c], "q": q_packed} for c in range(NCORES)]
    res = run_bass_kernel_spmd(nc, in_maps, list(range(NCORES)))
    return np.concatenate(
        [res.results[c]["cand"] for c in range(NCORES)], axis=1)


def _run_emulated(g_shards, q_packed):
    qf = q_packed.astype(np.float32).reshape(256, B)
    out = []
    for c in range(NCORES):
        gf = g_shards[c].astype(np.float32).reshape(256, NPC_PAD)
        sim = qf.T @ gf                                   # [B, NPC_PAD]
        res = np.empty((B, NREG * 8), np.float32)
        for r in range(NREG):
            a = r * 2048
            b = min(a + 2048, NPC_PAD)
            blkv = sim[:, a:b]
            top8 = -np.sort(-blkv, axis=1)[:, :8]
            res[:, r * 8:(r + 1) * 8] = top8
        out.append(res)
    return np.concatenate(out, axis=1)


def kernel(test_features, train_features, train_labels):
    test_features = np.asarray(test_features, dtype=np.float32)
    train_features = np.asarray(train_features, dtype=np.float32)
    train_labels = np.asarray(train_labels)

    import ml_dtypes
    bf16 = ml_dtypes.bfloat16

    # ---- host pre: fold normalizations into the query side ----
    tf64 = train_features.astype(np.float64)
    norm_d = np.maximum(np.sqrt(np.sum(tf64 * tf64, axis=0)), EPS)
    q64 = test_features.astype(np.float64)
    qn = np.sqrt(np.sum(q64 * q64, axis=1, keepdims=True))
    q_scaled = q64 / np.maximum(qn, EPS) / norm_d          # [B, D] f64

    q_packed = np.ascontiguousarray(
        q_scaled.T.astype(bf16).reshape(2, 128, B))
    gt = train_features.T.astype(bf16)                     # [D, N]
    g_shards = []
    for c in range(NCORES):
        sl = np.zeros((256, NPC_PAD), dtype=bf16)
        sl[:, :NPC] = gt[:, c * NPC:(c + 1) * NPC]
        g_shards.append(np.ascontiguousarray(sl.reshape(2, 128, NPC_PAD)))

    # ---- device: bf16 matmul + per-region top-8 screen ----
    if os.environ.get("KNN_EMULATE"):
        cand = _run_emulated(g_shards, q_packed)
    else:
        cand = _run_device(g_shards, q_packed)
    cand = cand.astype(np.float32)                         # [B, 8*NREG*8]

    # ---- host post: screen -> exact f64 rerank -> softmax scores ----
    topj = np.argpartition(-cand, TOPJ - 1, axis=1)[:, :TOPJ]
    reg_id = topj // 8                                     # 0..199 global

    reg_queries = {}
    for b in range(B):
        for r in set(reg_id[b].tolist()):
            reg_queries.setdefault(r, []).append(b)

    per_q_vals = [[] for _ in range(B)]
    per_q_cols = [[] for _ in range(B)]
    for r, qs in reg_queries.items():
        core, rc = divmod(r, NREG)
        c0 = core * NPC + rc * 2048
        c1 = core * NPC + min(rc * 2048 + 2048, NPC)
        block = tf64[c0:c1]                                # [w, D] view
        sims = q_scaled[qs] @ block.T                      # [nq, w] f64
        cols = np.arange(c0, c1)
        for i, b in enumerate(qs):
            per_q_vals[b].append(sims[i])
            per_q_cols[b].append(cols)

    labels = train_labels.astype(np.int64)
    scores = np.zeros((B, NUM_CLASSES), dtype=np.float64)
    for b in range(B):
        v = np.concatenate(per_q_vals[b])
        cidx = np.concatenate(per_q_cols[b])
        sel = np.argpartition(-v, NB_KNN - 1)[:NB_KNN]
        order = np.lexsort((cidx[sel], -v[sel]))
        sel = sel[order]
        topv = v[sel]
        w = np.exp(topv / T - np.max(topv) / T)
        w /= w.sum()
        np.add.at(scores[b], labels[cidx[sel]], w)
    return scores.astype(np.float32)


if __name__ == "__main__":
    rng = np.random.default_rng(0)
    tf = rng.standard_normal((B, D), dtype=np.float32)
    trf = rng.standard_normal((N, D), dtype=np.float32)
    trl = rng.integers(0, NUM_CLASSES, N).astype(np.int64)
    os.environ["KNN_EMULATE"] = "1"
    out = kernel(tf, trf, trl)
    print(out.shape, out.dtype, out.sum())
